# revision 1
# baseline (speedup 1.0000x reference)
"""MlpRBF kernel for 8 Trainium2 NeuronCores.

Math note: in this problem every MLP bias is zero and the MLP input is
x = |diff| >= 0.  Since relu(x*v) = x*relu(v) for scalar x >= 0, the whole
4-layer MLP collapses exactly (in exact arithmetic) to y = c * x with
    c = | relu(relu(relu(w0) @ w1) @ w2) @ w3 |   (scalar, host-computed).
So per element:  weight = |d| * (|d| < 0.25) * c
                 density[b,q] = sum_k weight[b,k,q]
                 out = weight / (density + 1e-5)

Device kernel per core (2 batches of the 16, data-parallel over batch):
  - tiles of [128 partitions, 8 k-rows, 512 q] (k = 8*p + ki packing keeps
    DMA fully contiguous in 2KB runs)
  - ACT: Y = |X|
  - DVE: W = (Y < 0.25) * Y              (one fused scalar_tensor_tensor)
  - PE : psum_d[1,512] += ones[128,1].T @ W[:,ki,:]   (partition reduction)
  - ACT: dens_row = c*psum_d ; d1 = c*psum_d + eps
  - DVE: r = reciprocal_approx_accurate(d1)           ([1,512])
  - PE : psum_R[128,512] = crow[1,128].T @ r[1,512]   (broadcast, = c*r)
  - ACT: R = copy(psum_R)
  - DVE/GPSIMD: OUT[:,ki,:] = W[:,ki,:] * R
"""

import numpy as np

import concourse.bacc as bacc
import concourse.bass as bass
import concourse.tile as tile
from concourse import mybir
from concourse.bass_utils import run_bass_kernel_spmd

F32 = mybir.dt.float32
AF = mybir.ActivationFunctionType
ALU = mybir.AluOpType

B, K, Q = 16, 1024, 1024
N_CORES = 8
BPC = B // N_CORES           # batches per core
KI = 8                       # k-rows packed per partition (128*8 = K)
QW = 512                     # q-tile width (PSUM bank limit for fp32 matmul)
QC = Q // QW                 # q-chunks per batch
WINDOW = 0.25
EPS = 1e-5

# how many of the KI final multiplies run on DVE (rest on GPSIMD)
N_FINAL_ON_DVE = 3

_NC = None


def _build_nc():
    nc = bacc.Bacc("TRN2", target_bir_lowering=False, debug=False,
                   num_devices=N_CORES)

    diff_h = nc.dram_tensor("diff", [BPC, K, Q], F32, kind="ExternalInput")
    cvec_h = nc.dram_tensor("cvec", [1, 128], F32, kind="ExternalInput")
    out_h = nc.dram_tensor("out", [BPC, K, Q], F32, kind="ExternalOutput")
    dens_h = nc.dram_tensor("density", [BPC, Q], F32, kind="ExternalOutput")

    with tile.TileContext(nc) as tc:
        with (
            tc.tile_pool(name="const", bufs=1) as constp,
            tc.tile_pool(name="xp", bufs=3) as xp,
            tc.tile_pool(name="yp", bufs=2) as yp,
            tc.tile_pool(name="wp", bufs=2) as wp,
            tc.tile_pool(name="op", bufs=2) as op_,
            tc.tile_pool(name="sp", bufs=2) as sp,
            tc.tile_pool(name="psd", bufs=2, space="PSUM") as psd,
            tc.tile_pool(name="psr", bufs=2, space="PSUM") as psr,
        ):
            ones_col = constp.tile([128, 1], F32)
            nc.any.memset(ones_col[:], 1.0)
            crow = constp.tile([1, 128], F32)
            nc.sync.dma_start(crow[:], cvec_h[:, :])

            for b in range(BPC):
                for qc in range(QC):
                    qs = qc * QW
                    src = diff_h[b, :, qs:qs + QW].rearrange(
                        "(p ki) j -> p ki j", ki=KI)
                    x = xp.tile([128, KI, QW], F32)
                    nc.sync.dma_start(x[:], src)

                    y = yp.tile([128, KI, QW], F32)
                    nc.scalar.activation(y[:], x[:], AF.Abs)

                    w = wp.tile([128, KI, QW], F32)
                    # w = (y < 0.25) * y  in one DVE pass
                    nc.vector.scalar_tensor_tensor(
                        w[:], y[:], WINDOW, y[:], ALU.is_lt, ALU.mult)

                    pd = psd.tile([1, QW], F32)
                    for ki in range(KI):
                        nc.tensor.matmul(pd[:], ones_col[:], w[:, ki, :],
                                         start=(ki == 0), stop=(ki == KI - 1))

                    # density output row = c * sum  (scale by c during copy)
                    drow = sp.tile([1, QW], F32, tag="drow")
                    nc.scalar.activation(drow[:], pd[:], AF.Copy,
                                         bias=0.0, scale=crow[:1, :1])
                    nc.scalar.dma_start(dens_h[b:b + 1, qs:qs + QW], drow[:])

                    # d1 = c*D + eps ; r = 1/d1 (~2 ulp)
                    d1 = sp.tile([1, QW], F32, tag="d1")
                    nc.scalar.activation(d1[:], pd[:], AF.Copy,
                                         bias=EPS, scale=crow[:1, :1])
                    rrow = sp.tile([1, QW], F32, tag="rrow")
                    scr = sp.tile([1, QW], F32, tag="scr")
                    nc.vector.reciprocal_approx_accurate(rrow[:], d1[:], scr[:])

                    # broadcast c*r across all 128 partitions via PE outer product
                    pR = psr.tile([128, QW], F32)
                    nc.tensor.matmul(pR[:], crow[:], rrow[:],
                                     start=True, stop=True)
                    rsb = sp.tile([128, QW], F32, tag="rsb")
                    nc.scalar.activation(rsb[:], pR[:], AF.Copy)

                    o = op_.tile([128, KI, QW], F32)
                    for ki in range(KI):
                        eng = nc.vector if ki < N_FINAL_ON_DVE else nc.gpsimd
                        eng.tensor_tensor(o[:, ki, :], w[:, ki, :], rsb[:],
                                          ALU.mult)

                    dst = out_h[b, :, qs:qs + QW].rearrange(
                        "(p ki) j -> p ki j", ki=KI)
                    nc.scalar.dma_start(dst, o[:])

    nc.compile()
    return nc


def get_nc():
    global _NC
    if _NC is None:
        _NC = _build_nc()
    return _NC


def compute_c(w0, w1, w2, w3):
    u = np.maximum(np.asarray(w0, np.float32), np.float32(0))
    v = np.maximum(u @ np.asarray(w1, np.float32), np.float32(0))
    t = np.maximum(v @ np.asarray(w2, np.float32), np.float32(0))
    return np.float32(abs((t @ np.asarray(w3, np.float32))[0, 0]))


def make_in_maps(diff, c):
    cvec = np.full((1, 128), c, dtype=np.float32)
    return [
        {"diff": np.ascontiguousarray(diff[i * BPC:(i + 1) * BPC]),
         "cvec": cvec}
        for i in range(N_CORES)
    ]


def kernel(**inputs):
    diff = np.asarray(inputs["diff"], dtype=np.float32)
    assert diff.shape == (B, K, Q), diff.shape
    c = compute_c(inputs["w0"], inputs["w1"], inputs["w2"], inputs["w3"])

    nc = get_nc()
    in_maps = make_in_maps(diff, c)
    res = run_bass_kernel_spmd(nc, in_maps, core_ids=list(range(N_CORES)))

    out = np.concatenate([r["out"] for r in res.results], axis=0)
    density = np.concatenate([r["density"] for r in res.results], axis=0)
    return out.astype(np.float32, copy=False), density.astype(np.float32,
                                                              copy=False)


# revision 3
# speedup vs baseline: 1.2688x; 1.2688x over previous
"""MlpRBF kernel for 8 Trainium2 NeuronCores.

Math note: in this problem every MLP bias is zero and the MLP input is
x = |diff| >= 0.  Since relu(x*v) = x*relu(v) for scalar x >= 0, the whole
4-layer MLP collapses exactly (in exact arithmetic) to y = c * x with
    c = | relu(relu(relu(w0) @ w1) @ w2) @ w3 |   (scalar, host-computed).
So per element:  weight = |d| * (|d| < 0.25) * c
                 density[b,q] = sum_k weight[b,k,q]
                 out = weight / (density + 1e-5)

Device kernel per core (2 batches of the 16, data-parallel over batch):
  - tiles of [128 partitions, 8 k-rows, 512 q] (k = 8*p + ki packing keeps
    DMA fully contiguous in 2KB runs)
  - ACT: Y = |X|
  - DVE: W = (Y < 0.25) * Y              (one fused scalar_tensor_tensor)
  - PE : psum_d[1,512] += ones[128,1].T @ W[:,ki,:]   (partition reduction)
  - ACT: dens_row = c*psum_d ; d1 = c*psum_d + eps
  - DVE: r = reciprocal_approx_accurate(d1)           ([1,512])
  - PE : psum_R[128,512] = crow[1,128].T @ r[1,512]   (broadcast, = c*r)
  - ACT: R = copy(psum_R)
  - DVE/GPSIMD: OUT[:,ki,:] = W[:,ki,:] * R
"""

import numpy as np

import concourse.bacc as bacc
import concourse.bass as bass
import concourse.tile as tile
from concourse import mybir
from concourse.bass_utils import run_bass_kernel_spmd

F32 = mybir.dt.float32
AF = mybir.ActivationFunctionType
ALU = mybir.AluOpType

B, K, Q = 16, 1024, 1024
N_CORES = 8
BPC = B // N_CORES           # batches per core
KI = 8                       # k-rows packed per partition (128*8 = K)
QW = 512                     # q-tile width (PSUM bank limit for fp32 matmul)
QC = Q // QW                 # q-chunks per batch
WINDOW = 0.25
EPS = 1e-5

_NC = None


def _build_nc():
    nc = bacc.Bacc("TRN2", target_bir_lowering=False, debug=False,
                   num_devices=N_CORES)

    diff_h = nc.dram_tensor("diff", [BPC, K, Q], F32, kind="ExternalInput")
    cvec_h = nc.dram_tensor("cvec", [1, 128], F32, kind="ExternalInput")
    out_h = nc.dram_tensor("out", [BPC, K, Q], F32, kind="ExternalOutput")
    dens_h = nc.dram_tensor("density", [BPC, Q], F32, kind="ExternalOutput")

    with tile.TileContext(nc) as tc:
        with (
            tc.tile_pool(name="const", bufs=1) as constp,
            tc.tile_pool(name="xp", bufs=3) as xp,
            tc.tile_pool(name="yp", bufs=2) as yp,
            tc.tile_pool(name="wp", bufs=2) as wp,
            tc.tile_pool(name="op", bufs=2) as op_,
            tc.tile_pool(name="sp", bufs=2) as sp,
            tc.tile_pool(name="psd", bufs=2, space="PSUM") as psd,
            tc.tile_pool(name="psr", bufs=2, space="PSUM") as psr,
        ):
            ones_col = constp.tile([128, 1], F32)
            nc.any.memset(ones_col[:], 1.0)
            crow = constp.tile([1, 128], F32)
            nc.sync.dma_start(crow[:], cvec_h[:, :])

            for b in range(BPC):
                for qc in range(QC):
                    qs = qc * QW
                    src = diff_h[b, :, qs:qs + QW].rearrange(
                        "(p ki) j -> p ki j", ki=KI)
                    x = xp.tile([128, KI, QW], F32)
                    nc.sync.dma_start(x[:], src)

                    # per-ki abs -> mask-mul -> matmul so ACT/DVE/PE overlap
                    y = yp.tile([128, KI, QW], F32)
                    w = wp.tile([128, KI, QW], F32)
                    pd = psd.tile([1, QW], F32)
                    for ki in range(KI):
                        nc.scalar.activation(y[:, ki, :], x[:, ki, :], AF.Abs)
                        # w = (y < 0.25) * y  in one DVE pass
                        nc.vector.scalar_tensor_tensor(
                            w[:, ki, :], y[:, ki, :], WINDOW, y[:, ki, :],
                            ALU.is_lt, ALU.mult)
                        nc.tensor.matmul(pd[:], ones_col[:], w[:, ki, :],
                                         start=(ki == 0), stop=(ki == KI - 1))

                    # density output row = c * sum  (scale by c during copy)
                    drow = sp.tile([1, QW], F32, tag="drow")
                    nc.scalar.activation(drow[:], pd[:], AF.Copy,
                                         bias=0.0, scale=crow[:1, :1])
                    nc.scalar.dma_start(dens_h[b:b + 1, qs:qs + QW], drow[:])

                    # d1 = c*D + eps ; r = 1/d1 (~51 ulp, fine vs 1e-4 scale)
                    d1 = sp.tile([1, QW], F32, tag="d1")
                    nc.scalar.activation(d1[:], pd[:], AF.Copy,
                                         bias=EPS, scale=crow[:1, :1])
                    rrow = sp.tile([1, QW], F32, tag="rrow")
                    nc.vector.reciprocal_approx_fast(rrow[:], d1[:])

                    # broadcast c*r across all 128 partitions via PE outer product
                    pR = psr.tile([128, QW], F32)
                    nc.tensor.matmul(pR[:], crow[:], rrow[:],
                                     start=True, stop=True)
                    rsb = sp.tile([128, QW], F32, tag="rsb")
                    nc.scalar.activation(rsb[:], pR[:], AF.Copy)

                    # final muls all on DVE (GPSIMD shares an SBUF port with
                    # DVE -- concurrent use slows both ~2x), stores chase the
                    # muls in 2-ki chunks so the store DMA packs tightly
                    o = op_.tile([128, KI, QW], F32)
                    dst = out_h[b, :, qs:qs + QW].rearrange(
                        "(p ki) j -> p ki j", ki=KI)
                    for ki in range(KI):
                        nc.vector.tensor_tensor(o[:, ki, :], w[:, ki, :],
                                                rsb[:], ALU.mult)
                        if ki % 2 == 1:
                            nc.scalar.dma_start(dst[:, ki - 1:ki + 1, :],
                                                o[:, ki - 1:ki + 1, :])

    nc.compile()
    return nc


def get_nc():
    global _NC
    if _NC is None:
        _NC = _build_nc()
    return _NC


def compute_c(w0, w1, w2, w3):
    u = np.maximum(np.asarray(w0, np.float32), np.float32(0))
    v = np.maximum(u @ np.asarray(w1, np.float32), np.float32(0))
    t = np.maximum(v @ np.asarray(w2, np.float32), np.float32(0))
    return np.float32(abs((t @ np.asarray(w3, np.float32))[0, 0]))


def make_in_maps(diff, c):
    cvec = np.full((1, 128), c, dtype=np.float32)
    return [
        {"diff": np.ascontiguousarray(diff[i * BPC:(i + 1) * BPC]),
         "cvec": cvec}
        for i in range(N_CORES)
    ]


def kernel(**inputs):
    diff = np.asarray(inputs["diff"], dtype=np.float32)
    assert diff.shape == (B, K, Q), diff.shape
    c = compute_c(inputs["w0"], inputs["w1"], inputs["w2"], inputs["w3"])

    nc = get_nc()
    in_maps = make_in_maps(diff, c)
    res = run_bass_kernel_spmd(nc, in_maps, core_ids=list(range(N_CORES)))

    out = np.concatenate([r["out"] for r in res.results], axis=0)
    density = np.concatenate([r["density"] for r in res.results], axis=0)
    return out.astype(np.float32, copy=False), density.astype(np.float32,
                                                              copy=False)


# revision 5
# speedup vs baseline: 1.4678x; 1.1568x over previous
"""MlpRBF kernel for 8 Trainium2 NeuronCores.

Math note: in this problem every MLP bias is zero and the MLP input is
x = |diff| >= 0.  Since relu(x*v) = x*relu(v) for scalar x >= 0, the whole
4-layer MLP collapses exactly (in exact arithmetic) to y = c * x with
    c = | relu(relu(relu(w0) @ w1) @ w2) @ w3 |   (scalar, host-computed).
So per element:  weight = |d| * (|d| < 0.25) * c
                 density[b,q] = sum_k weight[b,k,q]
                 out = weight / (density + 1e-5)

Device kernel per core (2 batches of the 16, data-parallel over batch):
  - tiles of [128 partitions, 8 k-rows, 512 q] (k = 8*p + ki packing keeps
    DMA fully contiguous in 2KB runs)
  - ACT: Y = |X|
  - DVE: W = (Y < 0.25) * Y              (one fused scalar_tensor_tensor)
  - PE : psum_d[1,512] += ones[128,1].T @ W[:,ki,:]   (partition reduction)
  - ACT: dens_row = c*psum_d ; d1 = c*psum_d + eps
  - DVE: r = reciprocal_approx_accurate(d1)           ([1,512])
  - PE : psum_R[128,512] = crow[1,128].T @ r[1,512]   (broadcast, = c*r)
  - ACT: R = copy(psum_R)
  - DVE/GPSIMD: OUT[:,ki,:] = W[:,ki,:] * R
"""

import numpy as np

import concourse.bacc as bacc
import concourse.bass as bass
import concourse.tile as tile
from concourse import mybir
from concourse.bass_utils import run_bass_kernel_spmd

F32 = mybir.dt.float32
AF = mybir.ActivationFunctionType
ALU = mybir.AluOpType

B, K, Q = 16, 1024, 1024
N_CORES = 8
BPC = B // N_CORES           # batches per core
KI = 8                       # k-rows packed per partition (128*8 = K)
QW = 512                     # q-tile width (PSUM bank limit for fp32 matmul)
QC = Q // QW                 # q-chunks per batch
WINDOW = 0.25
EPS = 1e-5

_NC = None


def _build_nc():
    nc = bacc.Bacc("TRN2", target_bir_lowering=False, debug=False,
                   num_devices=N_CORES)

    diff_h = nc.dram_tensor("diff", [BPC, K, Q], F32, kind="ExternalInput")
    cvec_h = nc.dram_tensor("cvec", [1, 128], F32, kind="ExternalInput")
    out_h = nc.dram_tensor("out", [BPC, K, Q], F32, kind="ExternalOutput")
    dens_h = nc.dram_tensor("density", [BPC, Q], F32, kind="ExternalOutput")

    with tile.TileContext(nc) as tc:
        with (
            tc.tile_pool(name="const", bufs=1) as constp,
            tc.tile_pool(name="xp", bufs=3) as xp,
            tc.tile_pool(name="yp", bufs=2) as yp,
            tc.tile_pool(name="wp", bufs=2) as wp,
            tc.tile_pool(name="op", bufs=2) as op_,
            tc.tile_pool(name="sp", bufs=2) as sp,
            tc.tile_pool(name="psd", bufs=2, space="PSUM") as psd,
            tc.tile_pool(name="psr", bufs=2, space="PSUM") as psr,
        ):
            ones_col = constp.tile([128, 1], F32)
            nc.any.memset(ones_col[:], 1.0)
            crow = constp.tile([1, 128], F32)
            nc.sync.dma_start(crow[:], cvec_h[:, :])

            for b in range(BPC):
                for qc in range(QC):
                    qs = qc * QW
                    src = diff_h[b, :, qs:qs + QW].rearrange(
                        "(p ki) j -> p ki j", ki=KI)
                    x = xp.tile([128, KI, QW], F32)
                    # split loads so compute starts after the first MB lands
                    nc.sync.dma_start(x[:, 0:KI // 2, :], src[:, 0:KI // 2, :])
                    nc.sync.dma_start(x[:, KI // 2:, :], src[:, KI // 2:, :])

                    # 2-ki chunks: abs (ACT) -> mask-mul (DVE) -> 2 matmuls
                    # (PE) so the three engines overlap inside a unit
                    y = yp.tile([128, KI, QW], F32)
                    w = wp.tile([128, KI, QW], F32)
                    pd = psd.tile([1, QW], F32)
                    for kh in range(KI // 2):
                        k0 = 2 * kh
                        nc.scalar.activation(y[:, k0:k0 + 2, :],
                                             x[:, k0:k0 + 2, :], AF.Abs)
                        # w = (y < 0.25) * y  in one DVE pass
                        nc.vector.scalar_tensor_tensor(
                            w[:, k0:k0 + 2, :], y[:, k0:k0 + 2, :], WINDOW,
                            y[:, k0:k0 + 2, :], ALU.is_lt, ALU.mult)
                        for ki in (k0, k0 + 1):
                            nc.tensor.matmul(pd[:], ones_col[:], w[:, ki, :],
                                             start=(ki == 0),
                                             stop=(ki == KI - 1))

                    # density output row = c * sum  (scale by c during copy)
                    drow = sp.tile([1, QW], F32, tag="drow")
                    nc.scalar.activation(drow[:], pd[:], AF.Copy,
                                         bias=0.0, scale=crow[:1, :1])
                    nc.gpsimd.dma_start(dens_h[b:b + 1, qs:qs + QW], drow[:])

                    # d1 = c*D + eps ; r = 1/d1 (~51 ulp, fine vs 1e-4 scale)
                    d1 = sp.tile([1, QW], F32, tag="d1")
                    nc.scalar.activation(d1[:], pd[:], AF.Copy,
                                         bias=EPS, scale=crow[:1, :1])
                    rrow = sp.tile([1, QW], F32, tag="rrow")
                    nc.vector.reciprocal_approx_fast(rrow[:], d1[:])

                    # broadcast c*r across all 128 partitions via PE outer product
                    pR = psr.tile([128, QW], F32)
                    nc.tensor.matmul(pR[:], crow[:], rrow[:],
                                     start=True, stop=True)
                    rsb = sp.tile([128, QW], F32, tag="rsb")
                    nc.scalar.activation(rsb[:], pR[:], AF.Copy)

                    # final muls all on DVE (GPSIMD SIMD work shares an SBUF
                    # port with DVE -- concurrent use slows both ~2x).
                    # stores go via SWDGE (gpsimd) so their trigger cost
                    # stays off the ACT/SP sequencers and they drain on a
                    # separate DMA queue from the loads.
                    o = op_.tile([128, KI, QW], F32)
                    dst = out_h[b, :, qs:qs + QW].rearrange(
                        "(p ki) j -> p ki j", ki=KI)
                    for kh in range(KI // 2):
                        k0 = 2 * kh
                        nc.vector.tensor_tensor(o[:, k0:k0 + 2, :],
                                                w[:, k0:k0 + 2, :],
                                                rsb[:].rearrange(
                                                    "p (a j) -> p a j", a=1
                                                ).broadcast_to([128, 2, QW]),
                                                ALU.mult)
                        nc.gpsimd.dma_start(dst[:, k0:k0 + 2, :],
                                            o[:, k0:k0 + 2, :])

    nc.compile()
    return nc


def get_nc():
    global _NC
    if _NC is None:
        _NC = _build_nc()
    return _NC


def compute_c(w0, w1, w2, w3):
    u = np.maximum(np.asarray(w0, np.float32), np.float32(0))
    v = np.maximum(u @ np.asarray(w1, np.float32), np.float32(0))
    t = np.maximum(v @ np.asarray(w2, np.float32), np.float32(0))
    return np.float32(abs((t @ np.asarray(w3, np.float32))[0, 0]))


def make_in_maps(diff, c):
    cvec = np.full((1, 128), c, dtype=np.float32)
    return [
        {"diff": np.ascontiguousarray(diff[i * BPC:(i + 1) * BPC]),
         "cvec": cvec}
        for i in range(N_CORES)
    ]


def kernel(**inputs):
    diff = np.asarray(inputs["diff"], dtype=np.float32)
    assert diff.shape == (B, K, Q), diff.shape
    c = compute_c(inputs["w0"], inputs["w1"], inputs["w2"], inputs["w3"])

    nc = get_nc()
    in_maps = make_in_maps(diff, c)
    res = run_bass_kernel_spmd(nc, in_maps, core_ids=list(range(N_CORES)))

    out = np.concatenate([r["out"] for r in res.results], axis=0)
    density = np.concatenate([r["density"] for r in res.results], axis=0)
    return out.astype(np.float32, copy=False), density.astype(np.float32,
                                                              copy=False)


# revision 12
# speedup vs baseline: 1.5272x; 1.0405x over previous
"""MlpRBF kernel for 8 Trainium2 NeuronCores.

Math note: in this problem every MLP bias is zero and the MLP input is
x = |diff| >= 0.  Since relu(x*v) = x*relu(v) for scalar x >= 0, the whole
4-layer MLP collapses exactly (in exact arithmetic) to y = c * x with
    c = | relu(relu(relu(w0) @ w1) @ w2) @ w3 |   (scalar, host-computed).
So per element:  weight = |d| * (|d| < 0.25) * c
                 density[b,q] = sum_k weight[b,k,q]
                 out = weight / (density + 1e-5)

Device kernel per core (2 batches of the 16, data-parallel over batch):
  - tiles of [128 partitions, 8 k-rows, 512 q] (k = 8*p + ki packing keeps
    DMA fully contiguous in 2KB runs)
  - ACT: Y = |X|                          (2-ki chunks)
  - DVE: W = (Y < 0.25) * Y               (one fused scalar_tensor_tensor)
  - PE : psum[128,512] += ones128.T @ W[:,ki,:]  -- lhsT of all-ones with
    M=128 makes every psum row the same K-partial sum, so the density
    arrives already broadcast across partitions (no separate bcast step)
  - ACT: d1 = c*psum + eps  (PSUM->SBUF, [128,512])
  - DVE: R = reciprocal_approx_fast(d1) * ... (R = 1/(c*D+eps))
  - DVE: OUT[:,ki,:] = W[:,ki,:] * (c*R)  -- c folded via ACT scale on d1?
    (we fold c into d1 so R = 1/(c*D+eps); the extra *c is folded into the
    mask-mul instead: W = (Y<0.25)*Y*c is NOT possible in one op, so we
    scale the abs: Y = |c*X| = c*|X| via ACT scale, and the mask threshold
    becomes c*0.25.)
"""

import numpy as np

import concourse.bacc as bacc
import concourse.bass as bass
import concourse.tile as tile
from concourse import mybir
from concourse.bass_utils import run_bass_kernel_spmd

F32 = mybir.dt.float32
AF = mybir.ActivationFunctionType
ALU = mybir.AluOpType

B, K, Q = 16, 1024, 1024
N_CORES = 8
BPC = B // N_CORES           # batches per core
KI = 8                       # k-rows packed per partition (128*8 = K)
QW = 512                     # q-tile width (PSUM bank limit for fp32 matmul)
QC = Q // QW                 # q-chunks per batch
WINDOW = 0.25
EPS = 1e-5

_NC = None


def _build_nc():
    nc = bacc.Bacc("TRN2", target_bir_lowering=False, debug=False,
                   num_devices=N_CORES)

    diff_h = nc.dram_tensor("diff", [BPC, K, Q], F32, kind="ExternalInput")
    # cvec[:,0] = c (runtime weight-derived scalar), cvec[:,1] = c*WINDOW
    cvec_h = nc.dram_tensor("cvec", [128, 2], F32, kind="ExternalInput")
    out_h = nc.dram_tensor("out", [BPC, K, Q], F32, kind="ExternalOutput")
    dens_h = nc.dram_tensor("density", [BPC, Q], F32, kind="ExternalOutput")

    with tile.TileContext(nc) as tc:
        with (
            tc.tile_pool(name="const", bufs=1) as constp,
            tc.tile_pool(name="xp", bufs=3) as xp,
            tc.tile_pool(name="yp", bufs=2) as yp,
            tc.tile_pool(name="wp", bufs=2) as wp,
            tc.tile_pool(name="op", bufs=2) as op_,
            tc.tile_pool(name="sp", bufs=2) as sp,
            tc.tile_pool(name="psd", bufs=2, space="PSUM") as psd,
        ):
            ones128 = constp.tile([128, 128], F32)
            nc.any.memset(ones128[:], 1.0)
            cv = constp.tile([128, 2], F32)
            nc.sync.dma_start(cv[:], cvec_h[:, :])

            for b in range(BPC):
                for qc in range(QC):
                    qs = qc * QW
                    src = diff_h[b, :, qs:qs + QW].rearrange(
                        "(p ki) j -> p ki j", ki=KI)
                    x = xp.tile([128, KI, QW], F32)
                    # split loads so compute starts after the first MB lands
                    nc.sync.dma_start(x[:, 0:KI // 2, :], src[:, 0:KI // 2, :])
                    nc.sync.dma_start(x[:, KI // 2:, :], src[:, KI // 2:, :])

                    # 2-ki chunks: abs (ACT) -> mask-mul (DVE) -> 2 matmuls
                    # (PE) so the three engines overlap inside a unit.
                    # NOTE: mask must compare the UNSCALED |x| against 0.25
                    # exactly as the reference does -- folding c in here
                    # could flip the mask for |x| within an ulp of 0.25.
                    y = yp.tile([128, KI, QW], F32)
                    w = wp.tile([128, KI, QW], F32)
                    pdR = psd.tile([128, QW], F32)
                    for kh in range(KI // 2):
                        k0 = 2 * kh
                        nc.scalar.activation(y[:, k0:k0 + 2, :],
                                             x[:, k0:k0 + 2, :], AF.Abs)
                        # w = (y < 0.25) * y  in one DVE pass
                        nc.vector.scalar_tensor_tensor(
                            w[:, k0:k0 + 2, :], y[:, k0:k0 + 2, :],
                            WINDOW, y[:, k0:k0 + 2, :],
                            ALU.is_lt, ALU.mult)
                        for ki in (k0, k0 + 1):
                            # all-ones [128,128] stationary: every psum row
                            # accumulates the same partition-sum -> density
                            # lands pre-broadcast across all 128 partitions
                            nc.tensor.matmul(pdR[:], ones128[:], w[:, ki, :],
                                             start=(ki == 0),
                                             stop=(ki == KI - 1))

                    # density output row = c * D  (psum holds raw D)
                    drow = sp.tile([1, QW], F32, tag="drow")
                    nc.scalar.activation(drow[:], pdR[0:1, :], AF.Copy,
                                         scale=cv[0:1, 0:1])
                    nc.gpsimd.dma_start(dens_h[b:b + 1, qs:qs + QW], drow[:])

                    # d1 = D + eps/c on all 128 partitions; then
                    # R = 1/d1 = c/(c*D + eps), which folds the final *c in
                    d1 = sp.tile([128, QW], F32, tag="d1")
                    nc.scalar.activation(d1[:], pdR[:], AF.Identity,
                                         bias=cv[:, 1:2])
                    rsb = sp.tile([128, QW], F32, tag="rsb")
                    nc.vector.reciprocal_approx_fast(rsb[:], d1[:])

                    # final muls all on DVE (GPSIMD SIMD work shares an SBUF
                    # port with DVE -- concurrent use slows both ~2x).
                    # stores go via SWDGE (gpsimd) so their trigger cost
                    # stays off the ACT/SP sequencers and they drain on a
                    # separate DMA queue from the loads.
                    o = op_.tile([128, KI, QW], F32)
                    dst = out_h[b, :, qs:qs + QW].rearrange(
                        "(p ki) j -> p ki j", ki=KI)
                    rb = rsb[:].rearrange("p (a j) -> p a j", a=1)
                    for kh in range(KI // 2):
                        k0 = 2 * kh
                        nc.vector.tensor_tensor(o[:, k0:k0 + 2, :],
                                                w[:, k0:k0 + 2, :],
                                                rb.broadcast_to([128, 2, QW]),
                                                ALU.mult)
                        nc.gpsimd.dma_start(dst[:, k0:k0 + 2, :],
                                            o[:, k0:k0 + 2, :])

    nc.compile()
    return nc


def get_nc():
    global _NC
    if _NC is None:
        _NC = _build_nc()
    return _NC


def compute_c(w0, w1, w2, w3):
    u = np.maximum(np.asarray(w0, np.float32), np.float32(0))
    v = np.maximum(u @ np.asarray(w1, np.float32), np.float32(0))
    t = np.maximum(v @ np.asarray(w2, np.float32), np.float32(0))
    return np.float32(abs((t @ np.asarray(w3, np.float32))[0, 0]))


def make_in_maps(diff, c):
    cvec = np.empty((128, 2), dtype=np.float32)
    cvec[:, 0] = c
    cvec[:, 1] = np.float32(np.float32(EPS) / np.float32(c))
    return [
        {"diff": np.ascontiguousarray(diff[i * BPC:(i + 1) * BPC]),
         "cvec": cvec}
        for i in range(N_CORES)
    ]


def kernel(**inputs):
    diff = np.asarray(inputs["diff"], dtype=np.float32)
    assert diff.shape == (B, K, Q), diff.shape
    c = compute_c(inputs["w0"], inputs["w1"], inputs["w2"], inputs["w3"])

    nc = get_nc()
    in_maps = make_in_maps(diff, c)
    res = run_bass_kernel_spmd(nc, in_maps, core_ids=list(range(N_CORES)))

    out = np.concatenate([r["out"] for r in res.results], axis=0)
    density = np.concatenate([r["density"] for r in res.results], axis=0)
    return out.astype(np.float32, copy=False), density.astype(np.float32,
                                                              copy=False)
